# revision 6
# baseline (speedup 1.0000x reference)
"""Trainium2 Bass kernel for nn_AutoEncoderLoss (two-level segment-mean MSE).

Strategy
--------
The loss needs per-(batch, cluster) sums of (reco-target)^2 and counts.
Counts depend only on the integer labels, so they are metadata computed on
the host while building the shard layout. For the float work, the host
chooses a *segment-sorted, column-aligned* layout: points are permuted so
each (batch, cluster) segment is contiguous and padded to a multiple of 128
(pad points have reco=target=0). Laid out column-major as [128, S], every
SBUF column then belongs to exactly one segment.

The device kernel is a pure streaming pipeline over [128, 2, S] fp8e4 input
(rec/tar at the middle axis):
  sub:    d = rec - tar  (bf16 out) — columns split between DVE and GpSimd
  square: v = d^2        (bf16)     — columns split between ScalarE and DVE
  sum:    PE ones-stationary matmul per 512-column chunk writes that chunk's
          per-column sums into its own PSUM row (psum[k, :] = sum_p v[p, :]).
A single DMA returns the [n_chunks, 512] PSUM block. The host bincounts
column sums into the [B*C] segment buffer (column -> segment map is host
metadata) and does the final masked two-level mean in float64.

fp8e4 input quantization adds ~0.14% bias to the loss (gate is 2e-2); the
heavy traffic is 2 * 1 byte/point, putting the kernel at the DMA roofline
(~17 KB/partition/core, ~7 us).
"""

import numpy as np
import ml_dtypes
from contextlib import ExitStack

NCORES = 8
CHUNK = 512          # PSUM bank columns (fp32) per chunk = one PSUM row
SUB_DVE_FRAC = 0.65  # fraction of sub columns on DVE (rest GpSimd)
SQ_DVE_FRAC = 0.15   # fraction of square columns on DVE (rest ScalarE)

_prog_cache = {}
_last_run = {}


def _tiles_for(S_pad):
    """Tile widths: small head (fill pipeline fast), big middle, small tail
    (short trailing compute chain). All multiples of CHUNK."""
    tiles = []
    rem = S_pad
    head = min(1024, rem)
    tiles.append(head)
    rem -= head
    while rem > 2048 + 1024:
        tiles.append(2048)
        rem -= 2048
    while rem > 512:
        w = min(1024, rem - 512)
        w = max(w // CHUNK, 1) * CHUNK
        tiles.append(w)
        rem -= w
    if rem:
        tiles.append(rem)
    out = []
    t0 = 0
    for w in tiles:
        out.append((t0, w))
        t0 += w
    assert t0 == S_pad
    return out


def _build_program(S_pad, repeat=None, internal_inputs=False):
    """SPMD program: [128, 2, S_pad] fp8e4 -> [n_chunks, 512] f32 col sums."""
    import concourse.tile as tile
    from concourse import bacc, mybir

    f32 = mybir.dt.float32
    bf16 = mybir.dt.bfloat16
    fp8 = mybir.dt.float8e4
    AT = mybir.ActivationFunctionType

    assert S_pad % CHUNK == 0
    nch = S_pad // CHUNK
    assert nch <= 128

    nc = bacc.Bacc("TRN2", target_bir_lowering=False, debug=False,
                   num_devices=NCORES)
    in_kind = "Internal" if internal_inputs else "ExternalInput"
    dat = nc.dram_tensor("dat", [128, 2, S_pad], fp8, kind=in_kind).ap()
    out = nc.dram_tensor("out", [nch, CHUNK], f32, kind="ExternalOutput").ap()

    tiles = _tiles_for(S_pad)

    with tile.TileContext(nc) as tc, ExitStack() as ctx:
        io_pool = ctx.enter_context(tc.tile_pool(name="io", bufs=3))
        tmp_pool = ctx.enter_context(tc.tile_pool(name="tmp", bufs=2))
        psum_pool = ctx.enter_context(tc.tile_pool(name="ps", bufs=1, space="PSUM"))
        const_pool = ctx.enter_context(tc.tile_pool(name="cst", bufs=1))

        # zeros|ones|zeros constant: sliding window k has its ones-column at
        # position k, routing chunk k's column sums to PSUM row k. (A plain
        # [128,1] ones stationary into ps[k:k+1] is not possible: PE tile
        # col offsets are restricted to {0,32,64,96}.)
        W = 2 * nch - 1
        ones_buf = const_pool.tile([128, W], bf16, tag="ones")
        nc.vector.memset(ones_buf[:], 0.0)
        nc.vector.memset(ones_buf[:, nch - 1:nch], 1.0)

        ps = psum_pool.tile([nch, CHUNK], f32, tag="ps", name="ps")

        if repeat is not None:
            ctx.enter_context(tc.For_i(0, repeat, 1))

        k = 0
        for i, (t0, tw) in enumerate(tiles):
            q = nc.sync if i % 2 == 0 else nc.scalar
            dt_ = io_pool.tile([128, 2, tw], fp8, tag="dat")
            q.dma_start(out=dt_[:], in_=dat[:, :, t0:t0 + tw])

            w1 = int(tw * SUB_DVE_FRAC / 16) * 16
            d = tmp_pool.tile([128, tw], bf16, tag="d")
            nc.vector.tensor_sub(d[:, :w1], dt_[:, 0, :w1], dt_[:, 1, :w1])
            nc.gpsimd.tensor_sub(d[:, w1:], dt_[:, 0, w1:], dt_[:, 1, w1:])

            w2 = int(tw * SQ_DVE_FRAC / 16) * 16
            v = tmp_pool.tile([128, tw], bf16, tag="v")
            if w2:
                nc.vector.tensor_mul(v[:, :w2], d[:, :w2], d[:, :w2])
            nc.scalar.activation(v[:, w2:], d[:, w2:], AT.Square)

            for j in range(tw // CHUNK):
                lhsT = ones_buf[:, nch - 1 - k:2 * nch - 1 - k]
                nc.tensor.matmul(ps[:], lhsT,
                                 v[:, j * CHUNK:(j + 1) * CHUNK],
                                 start=(k == 0), stop=(k == nch - 1))
                k += 1

        ob = const_pool.tile([nch, CHUNK], f32, tag="ob")
        nc.vector.tensor_copy(ob[:], ps[:])
        nc.sync.dma_start(out=out[:], in_=ob[:])

    nc.compile()
    return nc


def _layout(reco, target, clabel, batch_index, B, C):
    """Segment-sorted column-aligned shard layout (all host metadata work).

    Returns per-core fp8 [128, 2, S_pad] buffers, the column->segment map,
    exact per-segment counts, and S_total/S_pad.
    """
    N = reco.shape[0]
    seg = (batch_index.astype(np.int32) * np.int32(C)
           + clabel.astype(np.int32))
    nseg = B * C
    counts = np.bincount(seg, minlength=nseg)
    pad_cols = (counts + 127) // 128            # columns per segment
    col_start = np.zeros(nseg, dtype=np.int64)
    np.cumsum(pad_cols[:-1], out=col_start[1:])
    S_total = int(pad_cols.sum())

    S_core = -(-S_total // NCORES)
    S_pad = -(-S_core // CHUNK) * CHUNK
    S_cap = NCORES * S_pad

    # stable counting sort by segment; rank of each point within its segment
    perm = np.argsort(seg, kind="stable")
    pt_start = np.zeros(nseg, dtype=np.int64)
    np.cumsum(counts[:-1], out=pt_start[1:])
    rank = np.empty(N, dtype=np.int64)
    rank[perm] = np.arange(N, dtype=np.int64) - np.repeat(pt_start, counts)
    dest = 128 * col_start[seg] + rank          # linear slot, column-major

    buf = np.zeros((2, S_cap * 128), dtype=np.float32)
    buf[0, dest] = reco
    buf[1, dest] = target
    # [2, S_cap, 128] -> [128, 2, S_cap], contiguous per core after slicing
    arr = np.ascontiguousarray(
        buf.reshape(2, S_cap, 128).transpose(2, 0, 1)
    ).astype(ml_dtypes.float8_e4m3fn)

    col_seg = np.repeat(np.arange(nseg, dtype=np.int64), pad_cols)
    in_maps = []
    for m in range(NCORES):
        dat = np.ascontiguousarray(arr[:, :, m * S_pad:(m + 1) * S_pad])
        in_maps.append({"dat": dat})
    return in_maps, col_seg, counts, S_total, S_pad


def kernel(reco, target, clabel, batch_index, num_batches, num_clusters):
    from concourse.bass_utils import run_bass_kernel_spmd

    B = int(num_batches)
    C = int(num_clusters)
    reco = np.asarray(reco, dtype=np.float32).reshape(-1)
    target = np.asarray(target, dtype=np.float32).reshape(-1)
    clabel = np.asarray(clabel).reshape(-1)
    batch_index = np.asarray(batch_index).reshape(-1)

    in_maps, col_seg, counts, S_total, S_pad = _layout(
        reco, target, clabel, batch_index, B, C)

    key = (S_pad,)
    if key not in _prog_cache:
        _prog_cache[key] = _build_program(S_pad)
    nc = _prog_cache[key]

    _last_run["key"] = key
    res = None
    last_err = None
    for _attempt in range(3):  # the device occasionally faults transiently
        try:
            res = run_bass_kernel_spmd(nc, in_maps, list(range(NCORES)))
            break
        except Exception as e:  # noqa: BLE001
            last_err = e
            import time as _time
            _time.sleep(2.0)
    if res is None:
        raise last_err

    colsums = np.concatenate(
        [res.results[m]["out"].reshape(-1) for m in range(NCORES)]
    )[:S_total].astype(np.float64)
    nseg = B * C
    sums = np.bincount(col_seg, weights=colsums, minlength=nseg)
    cnt = counts.astype(np.float64)

    present = cnt > 0
    means = np.where(present, sums / np.where(present, cnt, 1.0), 0.0)
    means = means.reshape(B, C)
    pmask = present.reshape(B, C).astype(np.float64)
    n_clusters_b = pmask.sum(axis=1)
    b_present = n_clusters_b > 0
    batch_loss = (means * pmask).sum(axis=1) / np.where(b_present, n_clusters_b, 1.0)
    n_b = b_present.sum()
    loss = np.where(b_present, batch_loss, 0.0).sum() / max(n_b, 1)
    return np.float32(loss)


def profile_hw(np_inputs=None, k1=4, k2=1004, pairs=10, verbose=False):
    """Measure steady-state HW ns per kernel iteration.

    Two hardware-loop variants (k1/k2 repeats, Internal-DRAM inputs) run in
    interleaved pairs; median per-pair difference / (k2-k1) cancels dispatch
    overhead and is robust to slow patches on the time-shared device.
    """
    import time
    from concourse.bass_utils import run_bass_kernel_spmd
    if not _last_run and np_inputs is not None:
        kernel(**np_inputs)
    (S_pad,) = _last_run["key"]

    ncs = {}
    for k in (k1, k2):
        ck = ("prof", S_pad, k)
        if ck not in _prog_cache:
            _prog_cache[ck] = _build_program(S_pad, repeat=k,
                                             internal_inputs=True)
        ncs[k] = _prog_cache[ck]

    def one(k):
        t0 = time.time()
        run_bass_kernel_spmd(ncs[k], [{} for _ in range(NCORES)],
                             list(range(NCORES)))
        return time.time() - t0

    one(k1)  # warm both NEFFs
    one(k2)
    diffs = []
    for _ in range(pairs):
        try:
            ta = one(k1)
            tb = one(k2)
        except Exception:  # transient device flake: skip pair
            time.sleep(2)
            continue
        diffs.append((tb - ta) / (k2 - k1) * 1e9)
    diffs.sort()
    if verbose:
        print("pair diffs (ns/iter):", [f"{d:.0f}" for d in diffs])
    return diffs[len(diffs) // 2] if diffs else float("nan")


# revision 19
# speedup vs baseline: 1.5661x; 1.5661x over previous
"""Trainium2 Bass kernel for nn_AutoEncoderLoss (two-level segment-mean MSE).

Strategy
--------
The loss needs per-(batch, cluster) sums of (reco-target)^2 and counts.
Counts depend only on the integer labels, so they are metadata computed on
the host while building the shard layout. For the float work, the host
chooses a *segment-sorted, column-aligned* layout: points are permuted so
each (batch, cluster) segment is contiguous and padded to a multiple of 128
(pad points have reco=target=0). Laid out column-major as [128, S], every
SBUF column then belongs to exactly one segment.

The device kernel is a pure streaming pipeline over [128, 2, S] fp8e4 input
(rec/tar at the middle axis):
  sub:    d = rec - tar  (bf16 out) — columns split between DVE and GpSimd
  square: v = d^2        (bf16)     — columns split between ScalarE and DVE
  sum:    PE ones-stationary matmul per 512-column chunk writes that chunk's
          per-column sums into its own PSUM row (psum[k, :] = sum_p v[p, :]).
A single DMA returns the [n_chunks, 512] PSUM block. The host bincounts
column sums into the [B*C] segment buffer (column -> segment map is host
metadata) and does the final masked two-level mean in float64.

fp8e4 input quantization adds ~0.14% bias to the loss (gate is 2e-2); the
heavy traffic is 2 * 1 byte/point, putting the kernel at the DMA roofline
(~17 KB/partition/core, ~7 us).
"""

import numpy as np
import ml_dtypes
from contextlib import ExitStack

NCORES = 8
CHUNK = 512          # PSUM bank columns (fp32) per chunk = one PSUM row
SUB_DVE_FRAC = 0.65  # fraction of sub columns on DVE (rest GpSimd)
SQ_DVE_FRAC = 0.15   # fraction of square columns on DVE (rest ScalarE)
IN_DTYPE = "fp8"     # "fp8" (float8e4) or "bf16" input encoding
DUAL_QUEUE = True    # alternate input DMAs between SP and Activation queues
USE_FUSED = True     # single custom-DVE op sq(rec - tar) instead of sub+square


def _register_sqdiff():
    """Register (once) the custom DVE op out = (in0 - in1)^2."""
    import concourse.dve_ops as dve_ops
    from concourse.dve_spec import Spec, Src0, Src1, sq, lower
    from concourse.dve_uop import DveOpSpec

    for op in dve_ops.OPS:
        if op.name == "SQDIFF_ANT":
            return op
    spec = Spec(body=sq(Src0 - Src1),
                reference=lambda in0, in1: (in0 - in1) ** 2)
    row = dve_ops._CUSTOM_DVE_ROW_BASE + len(dve_ops.OPS)
    assert row < 0x20
    shas = {}
    for ver in ("v3", "v4"):
        s = DveOpSpec(name="SQDIFF_ANT", opcode=row,
                      uops=lower(spec, ver=ver), rd1_en=True)
        shas[ver] = s.sha(ver)
    op = dve_ops.DveOp("SQDIFF_ANT", spec, subdim=False, uops_sha=shas)
    dve_ops.OPS.append(op)
    dve_ops._SUB_OPCODE_FOR_NAME[op.name] = row
    dve_ops.CUSTOM_DVE_SPECS[op.name] = op.spec
    return op

_prog_cache = {}
_last_run = {}


def _tiles_for(S_pad):
    """Tile widths: small head (fill pipeline fast), big middle, small tail
    (short trailing compute chain). All multiples of CHUNK."""
    tiles = []
    rem = S_pad
    head = min(1024, rem)
    tiles.append(head)
    rem -= head
    while rem > 2048 + 1024:
        tiles.append(2048)
        rem -= 2048
    while rem > 512:
        w = min(1024, rem - 512)
        w = max(w // CHUNK, 1) * CHUNK
        tiles.append(w)
        rem -= w
    if rem:
        tiles.append(rem)
    out = []
    t0 = 0
    for w in tiles:
        out.append((t0, w))
        t0 += w
    assert t0 == S_pad
    return out


def _build_program(S_pad, repeat=None, internal_inputs=False):
    """SPMD program: [128, 2, S_pad] fp8e4 -> [n_chunks, 512] f32 col sums."""
    import concourse.tile as tile
    from concourse import bacc, mybir

    f32 = mybir.dt.float32
    bf16 = mybir.dt.bfloat16
    in_dt = mybir.dt.float8e4 if IN_DTYPE == "fp8" else bf16
    AT = mybir.ActivationFunctionType

    assert S_pad % CHUNK == 0
    nch = S_pad // CHUNK
    assert nch <= 128

    sqdiff = _register_sqdiff() if USE_FUSED else None
    nc = bacc.Bacc("TRN2", target_bir_lowering=False, debug=False,
                   num_devices=NCORES)
    in_kind = "Internal" if internal_inputs else "ExternalInput"
    dat = nc.dram_tensor("dat", [128, 2, S_pad], in_dt, kind=in_kind).ap()
    out = nc.dram_tensor("out", [nch, CHUNK], f32, kind="ExternalOutput").ap()

    tiles = _tiles_for(S_pad)

    with tile.TileContext(nc) as tc, ExitStack() as ctx:
        io_pool = ctx.enter_context(tc.tile_pool(name="io", bufs=3))
        tmp_pool = ctx.enter_context(tc.tile_pool(name="tmp", bufs=2))
        psum_pool = ctx.enter_context(tc.tile_pool(name="ps", bufs=1, space="PSUM"))
        const_pool = ctx.enter_context(tc.tile_pool(name="cst", bufs=1))

        # zeros|ones|zeros constant: sliding window k has its ones-column at
        # position k, routing chunk k's column sums to PSUM row k. (A plain
        # [128,1] ones stationary into ps[k:k+1] is not possible: PE tile
        # col offsets are restricted to {0,32,64,96}.)
        W = 2 * nch - 1
        ones_buf = const_pool.tile([128, W], bf16, tag="ones")
        nc.vector.memset(ones_buf[:], 0.0)
        nc.vector.memset(ones_buf[:, nch - 1:nch], 1.0)

        ps = psum_pool.tile([nch, CHUNK], f32, tag="ps", name="ps")

        if repeat is not None:
            ctx.enter_context(tc.For_i(0, repeat, 1))

        k = 0
        for i, (t0, tw) in enumerate(tiles):
            q = nc.sync if (i % 2 == 0 or not DUAL_QUEUE) else nc.scalar
            dt_ = io_pool.tile([128, 2, tw], in_dt, tag="dat")
            q.dma_start(out=dt_[:], in_=dat[:, :, t0:t0 + tw])

            v = tmp_pool.tile([128, tw], bf16, tag="v")
            if USE_FUSED:
                nc.vector._custom_dve(sqdiff, out=v[:], in0=dt_[:, 0, :],
                                      in1=dt_[:, 1, :])
            else:
                w1 = min(int(tw * SUB_DVE_FRAC / 16) * 16, tw)
                d = tmp_pool.tile([128, tw], bf16, tag="d")
                if w1:
                    nc.vector.tensor_sub(d[:, :w1], dt_[:, 0, :w1],
                                         dt_[:, 1, :w1])
                if w1 < tw:
                    nc.gpsimd.tensor_sub(d[:, w1:], dt_[:, 0, w1:],
                                         dt_[:, 1, w1:])

                w2 = min(int(tw * SQ_DVE_FRAC / 16) * 16, tw)
                if w2:
                    nc.vector.tensor_mul(v[:, :w2], d[:, :w2], d[:, :w2])
                if w2 < tw:
                    nc.scalar.activation(v[:, w2:], d[:, w2:], AT.Square)

            for j in range(tw // CHUNK):
                lhsT = ones_buf[:, nch - 1 - k:2 * nch - 1 - k]
                nc.tensor.matmul(ps[:], lhsT,
                                 v[:, j * CHUNK:(j + 1) * CHUNK],
                                 start=(k == 0), stop=(k == nch - 1))
                k += 1

        ob = const_pool.tile([nch, CHUNK], f32, tag="ob")
        if USE_FUSED:
            nc.scalar.copy(ob[:], ps[:])   # ScalarE is idle in fused mode
        else:
            nc.vector.tensor_copy(ob[:], ps[:])
        nc.sync.dma_start(out=out[:], in_=ob[:])

    nc.compile()
    return nc


def _layout(reco, target, clabel, batch_index, B, C):
    """Segment-sorted column-aligned shard layout (all host metadata work).

    Returns per-core fp8 [128, 2, S_pad] buffers, the column->segment map,
    exact per-segment counts, and S_total/S_pad.
    """
    N = reco.shape[0]
    seg = (batch_index.astype(np.int32) * np.int32(C)
           + clabel.astype(np.int32))
    nseg = B * C
    counts = np.bincount(seg, minlength=nseg)
    pad_cols = (counts + 127) // 128            # columns per segment
    col_start = np.zeros(nseg, dtype=np.int64)
    np.cumsum(pad_cols[:-1], out=col_start[1:])
    S_total = int(pad_cols.sum())

    S_core = -(-S_total // NCORES)
    S_pad = -(-S_core // CHUNK) * CHUNK
    S_cap = NCORES * S_pad

    # stable counting sort by segment; rank of each point within its segment
    perm = np.argsort(seg, kind="stable")
    pt_start = np.zeros(nseg, dtype=np.int64)
    np.cumsum(counts[:-1], out=pt_start[1:])
    rank = np.empty(N, dtype=np.int64)
    rank[perm] = np.arange(N, dtype=np.int64) - np.repeat(pt_start, counts)
    dest = 128 * col_start[seg] + rank          # linear slot, column-major

    buf = np.zeros((2, S_cap * 128), dtype=np.float32)
    buf[0, dest] = reco
    buf[1, dest] = target
    # [2, S_cap, 128] -> [128, 2, S_cap], contiguous per core after slicing
    np_dt = (ml_dtypes.float8_e4m3fn if IN_DTYPE == "fp8"
             else ml_dtypes.bfloat16)
    arr = np.ascontiguousarray(
        buf.reshape(2, S_cap, 128).transpose(2, 0, 1)
    ).astype(np_dt)

    col_seg = np.repeat(np.arange(nseg, dtype=np.int64), pad_cols)
    in_maps = []
    for m in range(NCORES):
        dat = np.ascontiguousarray(arr[:, :, m * S_pad:(m + 1) * S_pad])
        in_maps.append({"dat": dat})
    return in_maps, col_seg, counts, S_total, S_pad


def kernel(reco, target, clabel, batch_index, num_batches, num_clusters):
    from concourse.bass_utils import run_bass_kernel_spmd

    B = int(num_batches)
    C = int(num_clusters)
    reco = np.asarray(reco, dtype=np.float32).reshape(-1)
    target = np.asarray(target, dtype=np.float32).reshape(-1)
    clabel = np.asarray(clabel).reshape(-1)
    batch_index = np.asarray(batch_index).reshape(-1)

    in_maps, col_seg, counts, S_total, S_pad = _layout(
        reco, target, clabel, batch_index, B, C)

    key = (S_pad,)
    if key not in _prog_cache:
        _prog_cache[key] = _build_program(S_pad)
    nc = _prog_cache[key]

    _last_run["key"] = key
    res = None
    last_err = None
    for _attempt in range(3):  # the device occasionally faults transiently
        try:
            res = run_bass_kernel_spmd(nc, in_maps, list(range(NCORES)))
            break
        except Exception as e:  # noqa: BLE001
            last_err = e
            import time as _time
            _time.sleep(2.0)
    if res is None:
        raise last_err

    colsums = np.concatenate(
        [res.results[m]["out"].reshape(-1) for m in range(NCORES)]
    )[:S_total].astype(np.float64)
    nseg = B * C
    sums = np.bincount(col_seg, weights=colsums, minlength=nseg)
    cnt = counts.astype(np.float64)

    present = cnt > 0
    means = np.where(present, sums / np.where(present, cnt, 1.0), 0.0)
    means = means.reshape(B, C)
    pmask = present.reshape(B, C).astype(np.float64)
    n_clusters_b = pmask.sum(axis=1)
    b_present = n_clusters_b > 0
    batch_loss = (means * pmask).sum(axis=1) / np.where(b_present, n_clusters_b, 1.0)
    n_b = b_present.sum()
    loss = np.where(b_present, batch_loss, 0.0).sum() / max(n_b, 1)
    return np.float32(loss)


def profile_hw(np_inputs=None, k1=4, k2=1004, pairs=10, verbose=False):
    """Measure steady-state HW ns per kernel iteration.

    Two hardware-loop variants (k1/k2 repeats, Internal-DRAM inputs) run in
    interleaved pairs; median per-pair difference / (k2-k1) cancels dispatch
    overhead and is robust to slow patches on the time-shared device.
    """
    import time
    from concourse.bass_utils import run_bass_kernel_spmd
    if not _last_run and np_inputs is not None:
        kernel(**np_inputs)
    (S_pad,) = _last_run["key"]

    ncs = {}
    for k in (k1, k2):
        ck = ("prof", S_pad, k)
        if ck not in _prog_cache:
            _prog_cache[ck] = _build_program(S_pad, repeat=k,
                                             internal_inputs=True)
        ncs[k] = _prog_cache[ck]

    def one(k):
        t0 = time.time()
        run_bass_kernel_spmd(ncs[k], [{} for _ in range(NCORES)],
                             list(range(NCORES)))
        return time.time() - t0

    one(k1)  # warm both NEFFs
    one(k2)
    diffs = []
    for _ in range(pairs):
        try:
            ta = one(k1)
            tb = one(k2)
        except Exception:  # transient device flake: skip pair
            time.sleep(2)
            continue
        diffs.append((tb - ta) / (k2 - k1) * 1e9)
    diffs.sort()
    if verbose:
        print("pair diffs (ns/iter):", [f"{d:.0f}" for d in diffs])
    return diffs[len(diffs) // 2] if diffs else float("nan")


# revision 26
# speedup vs baseline: 1.6270x; 1.0388x over previous
"""Trainium2 Bass kernel for nn_AutoEncoderLoss (two-level segment-mean MSE).

Strategy
--------
The loss needs per-(batch, cluster) sums of (reco-target)^2 and counts.
Counts depend only on the integer labels, so they are metadata computed on
the host while building the shard layout. For the float work, the host
chooses a *segment-sorted, column-aligned* layout: points are permuted so
each (batch, cluster) segment is contiguous and padded to a multiple of 128
(pad points have reco=target=0). Laid out column-major as [128, S], every
SBUF column then belongs to exactly one segment.

The device kernel is a pure streaming pipeline over [128, 2, S] fp8e4 input
(rec/tar at the middle axis):
  sub:    d = rec - tar  (bf16 out) — columns split between DVE and GpSimd
  square: v = d^2        (bf16)     — columns split between ScalarE and DVE
  sum:    PE ones-stationary matmul per 512-column chunk writes that chunk's
          per-column sums into its own PSUM row (psum[k, :] = sum_p v[p, :]).
A single DMA returns the [n_chunks, 512] PSUM block. The host bincounts
column sums into the [B*C] segment buffer (column -> segment map is host
metadata) and does the final masked two-level mean in float64.

fp8e4 input quantization adds ~0.14% bias to the loss (gate is 2e-2); the
heavy traffic is 2 * 1 byte/point, putting the kernel at the DMA roofline
(~17 KB/partition/core, ~7 us).
"""

import numpy as np
import ml_dtypes
from contextlib import ExitStack

NCORES = 8
CHUNK = 512          # PSUM bank columns (fp32) per chunk = one PSUM row
SUB_DVE_FRAC = 0.65  # fraction of sub columns on DVE (rest GpSimd)
SQ_DVE_FRAC = 0.15   # fraction of square columns on DVE (rest ScalarE)
IN_DTYPE = "fp8"     # "fp8" (float8e4) or "bf16" input encoding
DUAL_QUEUE = False   # alternate input DMAs between SP and Activation queues
                     # (measured: SP-only is faster; Act-issued DMAs hurt)
USE_FUSED = True     # single custom-DVE op sq(rec - tar) instead of sub+square


def _register_sqdiff():
    """Register (once) the custom DVE op out = (in0 - in1)^2."""
    import concourse.dve_ops as dve_ops
    from concourse.dve_spec import Spec, Src0, Src1, sq, lower
    from concourse.dve_uop import DveOpSpec

    for op in dve_ops.OPS:
        if op.name == "SQDIFF_ANT":
            return op
    spec = Spec(body=sq(Src0 - Src1),
                reference=lambda in0, in1: (in0 - in1) ** 2)
    row = dve_ops._CUSTOM_DVE_ROW_BASE + len(dve_ops.OPS)
    assert row < 0x20
    shas = {}
    for ver in ("v3", "v4"):
        s = DveOpSpec(name="SQDIFF_ANT", opcode=row,
                      uops=lower(spec, ver=ver), rd1_en=True)
        shas[ver] = s.sha(ver)
    op = dve_ops.DveOp("SQDIFF_ANT", spec, subdim=False, uops_sha=shas)
    dve_ops.OPS.append(op)
    dve_ops._SUB_OPCODE_FOR_NAME[op.name] = row
    dve_ops.CUSTOM_DVE_SPECS[op.name] = op.spec
    return op

_prog_cache = {}
_last_run = {}


def _tiles_for(S_pad):
    """Tile widths: small head (fill pipeline fast), big middle, small tail
    (short trailing compute chain). All multiples of CHUNK."""
    tiles = []
    rem = S_pad
    head = min(1024, rem)
    tiles.append(head)
    rem -= head
    while rem > 2048 + 1024:
        tiles.append(2048)
        rem -= 2048
    while rem > 512:
        w = min(1024, rem - 512)
        w = max(w // CHUNK, 1) * CHUNK
        tiles.append(w)
        rem -= w
    if rem:
        tiles.append(rem)
    out = []
    t0 = 0
    for w in tiles:
        out.append((t0, w))
        t0 += w
    assert t0 == S_pad
    return out


def _build_program(S_pad, repeat=None, internal_inputs=False):
    """SPMD program: [128, 2, S_pad] fp8e4 -> [n_chunks, 512] f32 col sums."""
    import concourse.tile as tile
    from concourse import bacc, mybir

    f32 = mybir.dt.float32
    bf16 = mybir.dt.bfloat16
    in_dt = mybir.dt.float8e4 if IN_DTYPE == "fp8" else bf16
    AT = mybir.ActivationFunctionType

    assert S_pad % CHUNK == 0
    nch = S_pad // CHUNK
    assert nch <= 128

    sqdiff = _register_sqdiff() if USE_FUSED else None
    nc = bacc.Bacc("TRN2", target_bir_lowering=False, debug=False,
                   num_devices=NCORES)
    in_kind = "Internal" if internal_inputs else "ExternalInput"
    dat = nc.dram_tensor("dat", [128, 2, S_pad], in_dt, kind=in_kind).ap()
    out = nc.dram_tensor("out", [nch, CHUNK], f32, kind="ExternalOutput").ap()

    tiles = _tiles_for(S_pad)

    with tile.TileContext(nc) as tc, ExitStack() as ctx:
        io_pool = ctx.enter_context(tc.tile_pool(name="io", bufs=3))
        tmp_pool = ctx.enter_context(tc.tile_pool(name="tmp", bufs=2))
        psum_pool = ctx.enter_context(tc.tile_pool(name="ps", bufs=1, space="PSUM"))
        const_pool = ctx.enter_context(tc.tile_pool(name="cst", bufs=1))

        # zeros|ones|zeros constant: sliding window k has its ones-column at
        # position k, routing chunk k's column sums to PSUM row k. (A plain
        # [128,1] ones stationary into ps[k:k+1] is not possible: PE tile
        # col offsets are restricted to {0,32,64,96}.) Chunks are split over
        # two PSUM tiles so the first block's copy-out overlaps the second
        # block's matmuls.
        nchA = (nch + 1) // 2
        nchB = nch - nchA
        W = 2 * max(nchA, nchB) - 1
        ones_buf = const_pool.tile([128, W], bf16, tag="ones")
        nc.vector.memset(ones_buf[:], 0.0)
        nc.vector.memset(ones_buf[:, max(nchA, nchB) - 1:max(nchA, nchB)], 1.0)

        psA = psum_pool.tile([nchA, CHUNK], f32, tag="psA", name="psA")
        psB = (psum_pool.tile([nchB, CHUNK], f32, tag="psB", name="psB")
               if nchB else None)

        if repeat is not None:
            ctx.enter_context(tc.For_i(0, repeat, 1))

        k = 0
        for i, (t0, tw) in enumerate(tiles):
            q = nc.sync if (i % 2 == 0 or not DUAL_QUEUE) else nc.scalar
            dt_ = io_pool.tile([128, 2, tw], in_dt, tag="dat")
            q.dma_start(out=dt_[:], in_=dat[:, :, t0:t0 + tw])

            v = tmp_pool.tile([128, tw], bf16, tag="v")
            if USE_FUSED:
                nc.vector._custom_dve(sqdiff, out=v[:], in0=dt_[:, 0, :],
                                      in1=dt_[:, 1, :])
            else:
                w1 = min(int(tw * SUB_DVE_FRAC / 16) * 16, tw)
                d = tmp_pool.tile([128, tw], bf16, tag="d")
                if w1:
                    nc.vector.tensor_sub(d[:, :w1], dt_[:, 0, :w1],
                                         dt_[:, 1, :w1])
                if w1 < tw:
                    nc.gpsimd.tensor_sub(d[:, w1:], dt_[:, 0, w1:],
                                         dt_[:, 1, w1:])

                w2 = min(int(tw * SQ_DVE_FRAC / 16) * 16, tw)
                if w2:
                    nc.vector.tensor_mul(v[:, :w2], d[:, :w2], d[:, :w2])
                if w2 < tw:
                    nc.scalar.activation(v[:, w2:], d[:, w2:], AT.Square)

            for j in range(tw // CHUNK):
                if k < nchA:
                    ps, n_blk, kk, first, last = (
                        psA, nchA, k, k == 0, k == nchA - 1)
                else:
                    ps, n_blk, kk, first, last = (
                        psB, nchB, k - nchA, k == nchA, k == nch - 1)
                base = max(nchA, nchB) - 1
                lhsT = ones_buf[:, base - kk:base - kk + n_blk]
                nc.tensor.matmul(ps[:], lhsT,
                                 v[:, j * CHUNK:(j + 1) * CHUNK],
                                 start=first, stop=last)
                k += 1

        cp = nc.scalar if USE_FUSED else nc.vector  # ScalarE idle when fused
        copy = cp.copy if USE_FUSED else cp.tensor_copy
        obA = const_pool.tile([nchA, CHUNK], f32, tag="obA")
        copy(obA[:], psA[:])  # fires as soon as chunk nchA-1 is done
        nc.sync.dma_start(out=out[:nchA, :], in_=obA[:])
        if nchB:
            obB = const_pool.tile([nchB, CHUNK], f32, tag="obB")
            copy(obB[:], psB[:])
            nc.sync.dma_start(out=out[nchA:, :], in_=obB[:])

    nc.compile()
    return nc


def _layout(reco, target, clabel, batch_index, B, C):
    """Segment-sorted column-aligned shard layout (all host metadata work).

    Returns per-core fp8 [128, 2, S_pad] buffers, the column->segment map,
    exact per-segment counts, and S_total/S_pad.
    """
    N = reco.shape[0]
    seg = (batch_index.astype(np.int32) * np.int32(C)
           + clabel.astype(np.int32))
    nseg = B * C
    counts = np.bincount(seg, minlength=nseg)
    pad_cols = (counts + 127) // 128            # columns per segment
    col_start = np.zeros(nseg, dtype=np.int64)
    np.cumsum(pad_cols[:-1], out=col_start[1:])
    S_total = int(pad_cols.sum())

    S_core = -(-S_total // NCORES)
    S_pad = -(-S_core // CHUNK) * CHUNK
    S_cap = NCORES * S_pad

    # stable counting sort by segment; rank of each point within its segment
    perm = np.argsort(seg, kind="stable")
    pt_start = np.zeros(nseg, dtype=np.int64)
    np.cumsum(counts[:-1], out=pt_start[1:])
    rank = np.empty(N, dtype=np.int64)
    rank[perm] = np.arange(N, dtype=np.int64) - np.repeat(pt_start, counts)
    dest = 128 * col_start[seg] + rank          # linear slot, column-major

    buf = np.zeros((2, S_cap * 128), dtype=np.float32)
    buf[0, dest] = reco
    buf[1, dest] = target
    # [2, S_cap, 128] -> [128, 2, S_cap], contiguous per core after slicing
    np_dt = (ml_dtypes.float8_e4m3fn if IN_DTYPE == "fp8"
             else ml_dtypes.bfloat16)
    arr = np.ascontiguousarray(
        buf.reshape(2, S_cap, 128).transpose(2, 0, 1)
    ).astype(np_dt)

    col_seg = np.repeat(np.arange(nseg, dtype=np.int64), pad_cols)
    in_maps = []
    for m in range(NCORES):
        dat = np.ascontiguousarray(arr[:, :, m * S_pad:(m + 1) * S_pad])
        in_maps.append({"dat": dat})
    return in_maps, col_seg, counts, S_total, S_pad


def kernel(reco, target, clabel, batch_index, num_batches, num_clusters):
    from concourse.bass_utils import run_bass_kernel_spmd

    B = int(num_batches)
    C = int(num_clusters)
    reco = np.asarray(reco, dtype=np.float32).reshape(-1)
    target = np.asarray(target, dtype=np.float32).reshape(-1)
    clabel = np.asarray(clabel).reshape(-1)
    batch_index = np.asarray(batch_index).reshape(-1)

    in_maps, col_seg, counts, S_total, S_pad = _layout(
        reco, target, clabel, batch_index, B, C)

    key = (S_pad,)
    if key not in _prog_cache:
        _prog_cache[key] = _build_program(S_pad)
    nc = _prog_cache[key]

    _last_run["key"] = key
    res = None
    last_err = None
    for _attempt in range(3):  # the device occasionally faults transiently
        try:
            res = run_bass_kernel_spmd(nc, in_maps, list(range(NCORES)))
            break
        except Exception as e:  # noqa: BLE001
            last_err = e
            import time as _time
            _time.sleep(2.0)
    if res is None:
        raise last_err

    colsums = np.concatenate(
        [res.results[m]["out"].reshape(-1) for m in range(NCORES)]
    )[:S_total].astype(np.float64)
    nseg = B * C
    sums = np.bincount(col_seg, weights=colsums, minlength=nseg)
    cnt = counts.astype(np.float64)

    present = cnt > 0
    means = np.where(present, sums / np.where(present, cnt, 1.0), 0.0)
    means = means.reshape(B, C)
    pmask = present.reshape(B, C).astype(np.float64)
    n_clusters_b = pmask.sum(axis=1)
    b_present = n_clusters_b > 0
    batch_loss = (means * pmask).sum(axis=1) / np.where(b_present, n_clusters_b, 1.0)
    n_b = b_present.sum()
    loss = np.where(b_present, batch_loss, 0.0).sum() / max(n_b, 1)
    return np.float32(loss)


def profile_hw(np_inputs=None, k1=4, k2=1004, pairs=10, verbose=False):
    """Measure steady-state HW ns per kernel iteration.

    Two hardware-loop variants (k1/k2 repeats, Internal-DRAM inputs) run in
    interleaved pairs; median per-pair difference / (k2-k1) cancels dispatch
    overhead and is robust to slow patches on the time-shared device.
    """
    import time
    from concourse.bass_utils import run_bass_kernel_spmd
    if not _last_run and np_inputs is not None:
        kernel(**np_inputs)
    (S_pad,) = _last_run["key"]

    ncs = {}
    for k in (k1, k2):
        ck = ("prof", S_pad, k)
        if ck not in _prog_cache:
            _prog_cache[ck] = _build_program(S_pad, repeat=k,
                                             internal_inputs=True)
        ncs[k] = _prog_cache[ck]

    def one(k):
        t0 = time.time()
        run_bass_kernel_spmd(ncs[k], [{} for _ in range(NCORES)],
                             list(range(NCORES)))
        return time.time() - t0

    one(k1)  # warm both NEFFs
    one(k2)
    diffs = []
    for _ in range(pairs):
        try:
            ta = one(k1)
            tb = one(k2)
        except Exception:  # transient device flake: skip pair
            time.sleep(2)
            continue
        diffs.append((tb - ta) / (k2 - k1) * 1e9)
    diffs.sort()
    if verbose:
        print("pair diffs (ns/iter):", [f"{d:.0f}" for d in diffs])
    return diffs[len(diffs) // 2] if diffs else float("nan")


# revision 28
# speedup vs baseline: 1.8633x; 1.1453x over previous
"""Trainium2 Bass kernel for nn_AutoEncoderLoss (two-level segment-mean MSE).

Strategy
--------
The loss needs per-(batch, cluster) sums of (reco-target)^2 and counts.
Counts depend only on the integer labels, so they are metadata computed on
the host while building the shard layout. For the float work, the host
chooses a *segment-sorted, column-aligned* layout: points are permuted so
each (batch, cluster) segment is contiguous and padded to a multiple of 128
(pad points have reco=target=0). Laid out column-major as [128, S], every
SBUF column then belongs to exactly one segment.

The device kernel is a pure streaming pipeline over [128, 2, S] fp8e4 input
(rec/tar at the middle axis):
  DVE:  v = (rec - tar)^2 in ONE pass via a custom fused DVE op
        (SQDIFF_ANT = sq(Src0 - Src1), registered at build time). This
        keeps ScalarE (measured ~2-3x slower than its cost model) and
        GpSimd (far slower) out of the hot path entirely.
  PE:   ones-stationary matmul per 512-column chunk writes that chunk's
        per-column sums into its own PSUM row: a sliding zeros|ones|zeros
        stationary window routes chunk k to row k. Chunks are split over
        two PSUM tiles so the first block's ScalarE copy-out + DMA overlap
        the second block's matmuls.
The host bincounts the returned column sums into the [B*C] segment buffer
(column -> segment map is host metadata) and does the final masked
two-level mean in float64.

fp8e4 input quantization adds ~0.14% bias to the loss (gate is 2e-2); the
heavy traffic is 2 * 1 byte/point (~17 KB/partition/core), with DVE's
fused pass (~9 us model) and DMA (~7 us model) as the co-pacers.
"""

import numpy as np
import ml_dtypes
from contextlib import ExitStack

NCORES = 8
CHUNK = 512          # PSUM bank columns (fp32) per chunk = one PSUM row
SUB_DVE_FRAC = 0.65  # fraction of sub columns on DVE (rest GpSimd)
SQ_DVE_FRAC = 0.15   # fraction of square columns on DVE (rest ScalarE)
IN_DTYPE = "fp8"     # "fp8" (float8e4) or "bf16" input encoding
DUAL_QUEUE = False   # alternate input DMAs between SP and Activation queues
                     # (measured: SP-only is faster; Act-issued DMAs hurt)
USE_FUSED = True     # single custom-DVE op sq(rec - tar) instead of sub+square


def _register_sqdiff():
    """Register (once) the custom DVE op out = (in0 - in1)^2."""
    import concourse.dve_ops as dve_ops
    from concourse.dve_spec import Spec, Src0, Src1, sq, lower
    from concourse.dve_uop import DveOpSpec

    for op in dve_ops.OPS:
        if op.name == "SQDIFF_ANT":
            return op
    spec = Spec(body=sq(Src0 - Src1),
                reference=lambda in0, in1: (in0 - in1) ** 2)
    row = dve_ops._CUSTOM_DVE_ROW_BASE + len(dve_ops.OPS)
    assert row < 0x20
    shas = {}
    for ver in ("v3", "v4"):
        s = DveOpSpec(name="SQDIFF_ANT", opcode=row,
                      uops=lower(spec, ver=ver), rd1_en=True)
        shas[ver] = s.sha(ver)
    op = dve_ops.DveOp("SQDIFF_ANT", spec, subdim=False, uops_sha=shas)
    dve_ops.OPS.append(op)
    dve_ops._SUB_OPCODE_FOR_NAME[op.name] = row
    dve_ops.CUSTOM_DVE_SPECS[op.name] = op.spec
    return op

_prog_cache = {}
_last_run = {}


T_TILE = 2048        # main streaming tile width (columns)


def _tiles_for(S_pad):
    """Tile widths: small head (fill pipeline fast), big middle, small tail
    (short trailing compute chain). All multiples of CHUNK."""
    tiles = []
    rem = S_pad
    head = min(1024, rem)
    tiles.append(head)
    rem -= head
    while rem > T_TILE + 1024:
        tiles.append(T_TILE)
        rem -= T_TILE
    while rem > 512:
        w = min(1024, rem - 512)
        w = max(w // CHUNK, 1) * CHUNK
        tiles.append(w)
        rem -= w
    if rem:
        tiles.append(rem)
    out = []
    t0 = 0
    for w in tiles:
        out.append((t0, w))
        t0 += w
    assert t0 == S_pad
    return out


def _build_program(S_pad, repeat=None, internal_inputs=False):
    """SPMD program: [128, 2, S_pad] fp8e4 -> [n_chunks, 512] f32 col sums."""
    import concourse.tile as tile
    from concourse import bacc, mybir

    f32 = mybir.dt.float32
    bf16 = mybir.dt.bfloat16
    in_dt = mybir.dt.float8e4 if IN_DTYPE == "fp8" else bf16
    AT = mybir.ActivationFunctionType

    assert S_pad % CHUNK == 0
    nch = S_pad // CHUNK
    assert nch <= 128

    sqdiff = _register_sqdiff() if USE_FUSED else None
    nc = bacc.Bacc("TRN2", target_bir_lowering=False, debug=False,
                   num_devices=NCORES)
    in_kind = "Internal" if internal_inputs else "ExternalInput"
    dat = nc.dram_tensor("dat", [128, 2, S_pad], in_dt, kind=in_kind).ap()
    out = nc.dram_tensor("out", [nch, CHUNK], f32, kind="ExternalOutput").ap()

    tiles = _tiles_for(S_pad)

    with tile.TileContext(nc) as tc, ExitStack() as ctx:
        io_pool = ctx.enter_context(tc.tile_pool(name="io", bufs=3))
        tmp_pool = ctx.enter_context(tc.tile_pool(name="tmp", bufs=2))
        psum_pool = ctx.enter_context(tc.tile_pool(name="ps", bufs=1, space="PSUM"))
        const_pool = ctx.enter_context(tc.tile_pool(name="cst", bufs=1))

        # zeros|ones|zeros constant: sliding window k has its ones-column at
        # position k, routing chunk k's column sums to PSUM row k. (A plain
        # [128,1] ones stationary into ps[k:k+1] is not possible: PE tile
        # col offsets are restricted to {0,32,64,96}.) Chunks are split over
        # two PSUM tiles so the first block's copy-out overlaps the second
        # block's matmuls.
        nchA = (nch + 1) // 2
        nchB = nch - nchA
        W = 2 * max(nchA, nchB) - 1
        ones_buf = const_pool.tile([128, W], bf16, tag="ones")
        nc.vector.memset(ones_buf[:], 0.0)
        nc.vector.memset(ones_buf[:, max(nchA, nchB) - 1:max(nchA, nchB)], 1.0)

        psA = psum_pool.tile([nchA, CHUNK], f32, tag="psA", name="psA")
        psB = (psum_pool.tile([nchB, CHUNK], f32, tag="psB", name="psB")
               if nchB else None)

        if repeat is not None:
            ctx.enter_context(tc.For_i(0, repeat, 1))

        k = 0
        for i, (t0, tw) in enumerate(tiles):
            q = nc.sync if (i % 2 == 0 or not DUAL_QUEUE) else nc.scalar
            dt_ = io_pool.tile([128, 2, tw], in_dt, tag="dat")
            q.dma_start(out=dt_[:], in_=dat[:, :, t0:t0 + tw])

            v = tmp_pool.tile([128, tw], bf16, tag="v")
            if USE_FUSED:
                nc.vector._custom_dve(sqdiff, out=v[:], in0=dt_[:, 0, :],
                                      in1=dt_[:, 1, :])
            else:
                w1 = min(int(tw * SUB_DVE_FRAC / 16) * 16, tw)
                d = tmp_pool.tile([128, tw], bf16, tag="d")
                if w1:
                    nc.vector.tensor_sub(d[:, :w1], dt_[:, 0, :w1],
                                         dt_[:, 1, :w1])
                if w1 < tw:
                    nc.gpsimd.tensor_sub(d[:, w1:], dt_[:, 0, w1:],
                                         dt_[:, 1, w1:])

                w2 = min(int(tw * SQ_DVE_FRAC / 16) * 16, tw)
                if w2:
                    nc.vector.tensor_mul(v[:, :w2], d[:, :w2], d[:, :w2])
                if w2 < tw:
                    nc.scalar.activation(v[:, w2:], d[:, w2:], AT.Square)

            for j in range(tw // CHUNK):
                if k < nchA:
                    ps, n_blk, kk, first, last = (
                        psA, nchA, k, k == 0, k == nchA - 1)
                else:
                    ps, n_blk, kk, first, last = (
                        psB, nchB, k - nchA, k == nchA, k == nch - 1)
                base = max(nchA, nchB) - 1
                lhsT = ones_buf[:, base - kk:base - kk + n_blk]
                nc.tensor.matmul(ps[:], lhsT,
                                 v[:, j * CHUNK:(j + 1) * CHUNK],
                                 start=first, stop=last)
                k += 1

        cp = nc.scalar if USE_FUSED else nc.vector  # ScalarE idle when fused
        copy = cp.copy if USE_FUSED else cp.tensor_copy
        obA = const_pool.tile([nchA, CHUNK], f32, tag="obA")
        copy(obA[:], psA[:])  # fires as soon as chunk nchA-1 is done
        nc.sync.dma_start(out=out[:nchA, :], in_=obA[:])
        if nchB:
            obB = const_pool.tile([nchB, CHUNK], f32, tag="obB")
            copy(obB[:], psB[:])
            nc.sync.dma_start(out=out[nchA:, :], in_=obB[:])

    nc.compile()
    return nc


def _layout(reco, target, clabel, batch_index, B, C):
    """Segment-sorted column-aligned shard layout (all host metadata work).

    Returns per-core fp8 [128, 2, S_pad] buffers, the column->segment map,
    exact per-segment counts, and S_total/S_pad.
    """
    N = reco.shape[0]
    seg = (batch_index.astype(np.int32) * np.int32(C)
           + clabel.astype(np.int32))
    nseg = B * C
    counts = np.bincount(seg, minlength=nseg)
    pad_cols = (counts + 127) // 128            # columns per segment
    col_start = np.zeros(nseg, dtype=np.int64)
    np.cumsum(pad_cols[:-1], out=col_start[1:])
    S_total = int(pad_cols.sum())

    S_core = -(-S_total // NCORES)
    S_pad = -(-S_core // CHUNK) * CHUNK
    S_cap = NCORES * S_pad

    # stable counting sort by segment; rank of each point within its segment
    perm = np.argsort(seg, kind="stable")
    pt_start = np.zeros(nseg, dtype=np.int64)
    np.cumsum(counts[:-1], out=pt_start[1:])
    rank = np.empty(N, dtype=np.int64)
    rank[perm] = np.arange(N, dtype=np.int64) - np.repeat(pt_start, counts)
    dest = 128 * col_start[seg] + rank          # linear slot, column-major

    buf = np.zeros((2, S_cap * 128), dtype=np.float32)
    buf[0, dest] = reco
    buf[1, dest] = target
    # [2, S_cap, 128] -> [128, 2, S_cap], contiguous per core after slicing
    np_dt = (ml_dtypes.float8_e4m3fn if IN_DTYPE == "fp8"
             else ml_dtypes.bfloat16)
    arr = np.ascontiguousarray(
        buf.reshape(2, S_cap, 128).transpose(2, 0, 1)
    ).astype(np_dt)

    col_seg = np.repeat(np.arange(nseg, dtype=np.int64), pad_cols)
    in_maps = []
    for m in range(NCORES):
        dat = np.ascontiguousarray(arr[:, :, m * S_pad:(m + 1) * S_pad])
        in_maps.append({"dat": dat})
    return in_maps, col_seg, counts, S_total, S_pad


def kernel(reco, target, clabel, batch_index, num_batches, num_clusters):
    from concourse.bass_utils import run_bass_kernel_spmd

    B = int(num_batches)
    C = int(num_clusters)
    reco = np.asarray(reco, dtype=np.float32).reshape(-1)
    target = np.asarray(target, dtype=np.float32).reshape(-1)
    clabel = np.asarray(clabel).reshape(-1)
    batch_index = np.asarray(batch_index).reshape(-1)

    in_maps, col_seg, counts, S_total, S_pad = _layout(
        reco, target, clabel, batch_index, B, C)

    key = (S_pad,)
    if key not in _prog_cache:
        _prog_cache[key] = _build_program(S_pad)
    nc = _prog_cache[key]

    _last_run["key"] = key
    res = None
    last_err = None
    for _attempt in range(3):  # the device occasionally faults transiently
        try:
            res = run_bass_kernel_spmd(nc, in_maps, list(range(NCORES)))
            break
        except Exception as e:  # noqa: BLE001
            last_err = e
            import time as _time
            _time.sleep(2.0)
    if res is None:
        raise last_err

    colsums = np.concatenate(
        [res.results[m]["out"].reshape(-1) for m in range(NCORES)]
    )[:S_total].astype(np.float64)
    nseg = B * C
    sums = np.bincount(col_seg, weights=colsums, minlength=nseg)
    cnt = counts.astype(np.float64)

    present = cnt > 0
    means = np.where(present, sums / np.where(present, cnt, 1.0), 0.0)
    means = means.reshape(B, C)
    pmask = present.reshape(B, C).astype(np.float64)
    n_clusters_b = pmask.sum(axis=1)
    b_present = n_clusters_b > 0
    batch_loss = (means * pmask).sum(axis=1) / np.where(b_present, n_clusters_b, 1.0)
    n_b = b_present.sum()
    loss = np.where(b_present, batch_loss, 0.0).sum() / max(n_b, 1)
    return np.float32(loss)


def profile_hw(np_inputs=None, k1=4, k2=1004, pairs=10, verbose=False):
    """Measure steady-state HW ns per kernel iteration.

    Two hardware-loop variants (k1/k2 repeats, Internal-DRAM inputs) run in
    interleaved pairs; median per-pair difference / (k2-k1) cancels dispatch
    overhead and is robust to slow patches on the time-shared device.
    """
    import time
    from concourse.bass_utils import run_bass_kernel_spmd
    if not _last_run and np_inputs is not None:
        kernel(**np_inputs)
    (S_pad,) = _last_run["key"]

    ncs = {}
    for k in (k1, k2):
        ck = ("prof", S_pad, k)
        if ck not in _prog_cache:
            _prog_cache[ck] = _build_program(S_pad, repeat=k,
                                             internal_inputs=True)
        ncs[k] = _prog_cache[ck]

    def one(k):
        t0 = time.time()
        run_bass_kernel_spmd(ncs[k], [{} for _ in range(NCORES)],
                             list(range(NCORES)))
        return time.time() - t0

    one(k1)  # warm both NEFFs
    one(k2)
    diffs = []
    for _ in range(pairs):
        try:
            ta = one(k1)
            tb = one(k2)
        except Exception:  # transient device flake: skip pair
            time.sleep(2)
            continue
        diffs.append((tb - ta) / (k2 - k1) * 1e9)
    diffs.sort()
    if verbose:
        print("pair diffs (ns/iter):", [f"{d:.0f}" for d in diffs])
    return diffs[len(diffs) // 2] if diffs else float("nan")


# revision 45
# speedup vs baseline: 2.3024x; 1.2356x over previous
"""Trainium2 Bass kernel for nn_AutoEncoderLoss (two-level segment-mean MSE).

Strategy
--------
The loss needs per-(batch, cluster) sums of (reco-target)^2 and counts.
Counts depend only on the integer labels, so they are metadata computed on
the host while building the shard layout. For the float work, the host
chooses a *segment-sorted, column-aligned* layout: points are permuted so
each (batch, cluster) segment is contiguous and padded to a multiple of 128
(pad points have reco=target=0). Laid out column-major as [128, S], every
SBUF column then belongs to exactly one segment.

The device kernel is a pure streaming pipeline over [128, 2, S] fp8e4 input
(rec/tar at the middle axis):
  DVE:  v = (rec - tar)^2 in ONE pass via a custom fused DVE op
        (SQDIFF_ANT = sq(Src0 - Src1), registered at build time). This
        keeps ScalarE (measured ~2-3x slower than its cost model) and
        GpSimd (far slower) out of the hot path entirely.
  PE:   ones-stationary matmul per 512-column chunk writes that chunk's
        per-column sums into its own PSUM row: a sliding zeros|ones|zeros
        stationary window routes chunk k to row k. Chunks are split over
        two PSUM tiles so the first block's ScalarE copy-out + DMA overlap
        the second block's matmuls.
The host bincounts the returned column sums into the [B*C] segment buffer
(column -> segment map is host metadata) and does the final masked
two-level mean in float64.

fp8e4 input quantization adds ~0.14% bias to the loss (gate is 2e-2); the
heavy traffic is 2 * 1 byte/point (~17 KB/partition/core), with DVE's
fused pass (~9 us model) and DMA (~7 us model) as the co-pacers.
"""

import numpy as np
import ml_dtypes
from contextlib import ExitStack

NCORES = 8
CHUNK = 512          # PSUM bank columns (fp32) per chunk = one PSUM row
SUB_DVE_FRAC = 0.65  # fraction of sub columns on DVE (rest GpSimd)
SQ_DVE_FRAC = 0.15   # fraction of square columns on DVE (rest ScalarE)
IN_DTYPE = "fp8"     # "fp8" (float8e4) or "bf16" input encoding
DUAL_QUEUE = False   # alternate input DMAs between SP and Activation queues
                     # (measured: SP-only is faster; Act-issued DMAs hurt)
USE_FUSED = True     # single custom-DVE op sq(rec - tar) instead of sub+square
USE_DR = True        # fp8 DoubleRow matmul: each 512-col chunk reduces to 256
                     # pair-sums (cols n, n+256 same segment by host layout);
                     # 4x fewer PE cycles. Requires stationary M multiple of 16.


def _register_sqdiff():
    """Register (once) the custom DVE op out = (in0 - in1)^2."""
    import concourse.dve_ops as dve_ops
    from concourse.dve_spec import Spec, Src0, Src1, sq, lower
    from concourse.dve_uop import DveOpSpec

    for op in dve_ops.OPS:
        if op.name == "SQDIFF_ANT":
            return op
    spec = Spec(body=sq(Src0 - Src1),
                reference=lambda in0, in1: (in0 - in1) ** 2)
    row = dve_ops._CUSTOM_DVE_ROW_BASE + len(dve_ops.OPS)
    assert row < 0x20
    shas = {}
    for ver in ("v3", "v4"):
        s = DveOpSpec(name="SQDIFF_ANT", opcode=row,
                      uops=lower(spec, ver=ver), rd1_en=True)
        shas[ver] = s.sha(ver)
    op = dve_ops.DveOp("SQDIFF_ANT", spec, subdim=False, uops_sha=shas)
    dve_ops.OPS.append(op)
    dve_ops._SUB_OPCODE_FOR_NAME[op.name] = row
    dve_ops.CUSTOM_DVE_SPECS[op.name] = op.spec
    return op

_prog_cache = {}
_last_run = {}


T_TILE = 2048        # main streaming tile width (columns)


def _tiles_for(S_pad):
    """Tile widths: small head (fill pipeline fast), big middle, small tail
    (short trailing compute chain). All multiples of CHUNK."""
    tiles = []
    rem = S_pad
    head = min(1024, rem)
    tiles.append(head)
    rem -= head
    while rem > T_TILE + 1024:
        tiles.append(T_TILE)
        rem -= T_TILE
    while rem > 512:
        w = min(1024, rem - 512)
        w = max(w // CHUNK, 1) * CHUNK
        tiles.append(w)
        rem -= w
    if rem:
        tiles.append(rem)
    out = []
    t0 = 0
    for w in tiles:
        out.append((t0, w))
        t0 += w
    assert t0 == S_pad
    return out


def _build_program(S_pad, repeat=None, internal_inputs=False, stage="full"):
    """SPMD program: [128, 2, S_pad] fp8e4 -> [n_chunks, 512] f32 col sums.

    stage: "dma" (loads only), "dve" (+fused square-diff), "pe" (+matmuls),
    "full" (+copy-out). Cut-down stages are for bottleneck isolation.
    """
    import concourse.tile as tile
    from concourse import bacc, mybir

    f32 = mybir.dt.float32
    bf16 = mybir.dt.bfloat16
    in_dt = mybir.dt.float8e4 if IN_DTYPE == "fp8" else bf16
    AT = mybir.ActivationFunctionType

    assert S_pad % CHUNK == 0
    nch = S_pad // CHUNK
    assert nch <= 128

    sqdiff = _register_sqdiff() if USE_FUSED else None
    nc = bacc.Bacc("TRN2", target_bir_lowering=False, debug=False,
                   num_devices=NCORES)
    in_kind = "Internal" if internal_inputs else "ExternalInput"
    dat = nc.dram_tensor("dat", [128, 2, S_pad], in_dt, kind=in_kind).ap()
    outw = CHUNK // 2 if USE_DR else CHUNK
    out = nc.dram_tensor("out", [nch, outw], f32, kind="ExternalOutput").ap()
    if USE_DR:
        assert IN_DTYPE == "fp8" and USE_FUSED

    tiles = _tiles_for(S_pad)

    with tile.TileContext(nc) as tc, ExitStack() as ctx:
        io_pool = ctx.enter_context(tc.tile_pool(name="io", bufs=3))
        tmp_pool = ctx.enter_context(tc.tile_pool(name="tmp", bufs=2))
        psum_pool = ctx.enter_context(tc.tile_pool(name="ps", bufs=1, space="PSUM"))
        const_pool = ctx.enter_context(tc.tile_pool(name="cst", bufs=1))

        # zeros|ones|zeros constant: sliding window k has its ones-column at
        # position k, routing chunk k's column sums to PSUM row k. (A plain
        # [128,1] ones stationary into ps[k:k+1] is not possible: PE tile
        # col offsets are restricted to {0,32,64,96}.) Chunks are split over
        # two PSUM tiles so the first block's copy-out overlaps the second
        # block's matmuls.
        use_pe = stage in ("pe", "full")
        if USE_DR:
            # DoubleRow: stationary must be [128, 2, M] fp8 with M a multiple
            # of 16 (ISA restriction); fixed M=16 blocks, unused rows just
            # accumulate zeros. Output per chunk = 256 pair-sums.
            M = 16
            assert nch <= 2 * M
            nchA = min(nch, M)
            nchB = nch - nchA
            OUTW = CHUNK // 2
            # one aligned [128, 2, M] stationary per row k (sliding-window
            # slices at 1-byte offsets fail s3_lw_dual_fp8_restrictions)
            ones_buf = const_pool.tile([128, M, 2, M], in_dt, tag="ones")
            nc.vector.memset(ones_buf[:], 0.0)
            for kk in range(M):
                nc.vector.memset(ones_buf[:, kk, :, kk:kk + 1], 1.0)
            psA = (psum_pool.tile([M, OUTW], f32, tag="psA", name="psA")
                   if use_pe else None)
            psB = (psum_pool.tile([M, OUTW], f32, tag="psB", name="psB")
                   if use_pe and nchB else None)
        else:
            nchA = (nch + 1) // 2
            nchB = nch - nchA
            OUTW = CHUNK
            W = 2 * max(nchA, nchB) - 1
            ones_buf = const_pool.tile([128, W], bf16, tag="ones")
            nc.vector.memset(ones_buf[:], 0.0)
            nc.vector.memset(
                ones_buf[:, max(nchA, nchB) - 1:max(nchA, nchB)], 1.0)
            psA = (psum_pool.tile([nchA, OUTW], f32, tag="psA", name="psA")
                   if use_pe else None)
            psB = (psum_pool.tile([nchB, OUTW], f32, tag="psB", name="psB")
                   if use_pe and nchB else None)

        if repeat is not None:
            ctx.enter_context(tc.For_i(0, repeat, 1))

        k = 0
        for i, (t0, tw) in enumerate(tiles):
            q = nc.sync if (i % 2 == 0 or not DUAL_QUEUE) else nc.scalar
            dt_ = io_pool.tile([128, 2, tw], in_dt, tag="dat")
            q.dma_start(out=dt_[:], in_=dat[:, :, t0:t0 + tw])
            if stage == "dma":
                continue

            v = tmp_pool.tile([128, tw], in_dt if USE_DR else bf16, tag="v")
            if USE_FUSED:
                nc.vector._custom_dve(sqdiff, out=v[:], in0=dt_[:, 0, :],
                                      in1=dt_[:, 1, :])
            else:
                w1 = min(int(tw * SUB_DVE_FRAC / 16) * 16, tw)
                d = tmp_pool.tile([128, tw], bf16, tag="d")
                if w1:
                    nc.vector.tensor_sub(d[:, :w1], dt_[:, 0, :w1],
                                         dt_[:, 1, :w1])
                if w1 < tw:
                    nc.gpsimd.tensor_sub(d[:, w1:], dt_[:, 0, w1:],
                                         dt_[:, 1, w1:])

                w2 = min(int(tw * SQ_DVE_FRAC / 16) * 16, tw)
                if w2:
                    nc.vector.tensor_mul(v[:, :w2], d[:, :w2], d[:, :w2])
                if w2 < tw:
                    nc.scalar.activation(v[:, w2:], d[:, w2:], AT.Square)

            if stage == "dve":
                continue
            for j in range(tw // CHUNK):
                if k < nchA:
                    ps, kk, first, last = psA, k, k == 0, k == nchA - 1
                else:
                    ps, kk, first, last = (
                        psB, k - nchA, k == nchA, k == nch - 1)
                if USE_DR:
                    lhsT = ones_buf[:, kk, :, :]  # aligned [128, 2, 16]
                    rhs = v[:, j * CHUNK:(j + 1) * CHUNK].rearrange(
                        "p (j2 n) -> p j2 n", j2=2)
                    nc.tensor.matmul(
                        ps[:], lhsT, rhs, start=first, stop=last,
                        perf_mode=mybir.MatmulPerfMode.DoubleRow)
                else:
                    n_blk = nchA if k < nchA else nchB
                    base = max(nchA, nchB) - 1
                    lhsT = ones_buf[:, base - kk:base - kk + n_blk]
                    nc.tensor.matmul(ps[:], lhsT,
                                     v[:, j * CHUNK:(j + 1) * CHUNK],
                                     start=first, stop=last)
                k += 1

        if stage == "full":
            cp = nc.scalar if USE_FUSED else nc.vector  # ScalarE idle if fused
            copy = cp.copy if USE_FUSED else cp.tensor_copy
            obA = const_pool.tile([nchA, outw], f32, tag="obA")
            copy(obA[:], psA[:nchA, :])  # fires once chunk nchA-1 is done
            nc.sync.dma_start(out=out[:nchA, :], in_=obA[:])
            if nchB:
                obB = const_pool.tile([nchB, outw], f32, tag="obB")
                copy(obB[:], psB[:nchB, :])
                nc.sync.dma_start(out=out[nchA:, :], in_=obB[:])

    nc.compile()
    return nc


def _layout(reco, target, clabel, batch_index, B, C):
    """Segment-sorted column-aligned shard layout (all host metadata work).

    Returns per-core fp8 [128, 2, S_pad] buffers, the column->segment map,
    exact per-segment counts, and S_total/S_pad.
    """
    N = reco.shape[0]
    seg = (batch_index.astype(np.int32) * np.int32(C)
           + clabel.astype(np.int32))
    nseg = B * C
    counts = np.bincount(seg, minlength=nseg)
    if USE_DR:
        # DoubleRow merges column pairs (n, n+256) of each 512-col chunk, so
        # segments must cover an even number of columns (256-point multiples).
        pad_cols = 2 * ((counts + 255) // 256)
    else:
        pad_cols = (counts + 127) // 128        # columns per segment
    col_start = np.zeros(nseg, dtype=np.int64)
    np.cumsum(pad_cols[:-1], out=col_start[1:])
    S_total = int(pad_cols.sum())

    S_core = -(-S_total // NCORES)
    S_pad = -(-S_core // CHUNK) * CHUNK
    S_cap = NCORES * S_pad

    # stable counting sort by segment; rank of each point within its segment
    perm = np.argsort(seg, kind="stable")
    pt_start = np.zeros(nseg, dtype=np.int64)
    np.cumsum(counts[:-1], out=pt_start[1:])
    rank = np.empty(N, dtype=np.int64)
    rank[perm] = np.arange(N, dtype=np.int64) - np.repeat(pt_start, counts)
    dest = 128 * col_start[seg] + rank          # linear slot, column-major

    buf = np.zeros((2, S_cap * 128), dtype=np.float32)
    buf[0, dest] = reco
    buf[1, dest] = target
    # [2, S_cap, 128] -> [128, 2, S_cap], contiguous per core after slicing
    np_dt = (ml_dtypes.float8_e4m3fn if IN_DTYPE == "fp8"
             else ml_dtypes.bfloat16)
    arr = np.ascontiguousarray(
        buf.reshape(2, S_cap, 128).transpose(2, 0, 1)
    ).astype(np_dt)

    if USE_DR:
        # logical col l = c*512 + 2m + j  ->  physical c*512 + j*256 + m, so
        # the device's pair (n, n+256) is the logical pair (2m, 2m+1).
        arr = np.ascontiguousarray(
            arr.reshape(128, 2, S_cap // CHUNK, CHUNK // 2, 2)
            .swapaxes(-1, -2)
            .reshape(128, 2, S_cap))
        unit_seg = np.repeat(np.arange(nseg, dtype=np.int64), pad_cols // 2)
        n_units = S_total // 2
    else:
        unit_seg = np.repeat(np.arange(nseg, dtype=np.int64), pad_cols)
        n_units = S_total

    in_maps = []
    for m in range(NCORES):
        dat = np.ascontiguousarray(arr[:, :, m * S_pad:(m + 1) * S_pad])
        in_maps.append({"dat": dat})
    return in_maps, unit_seg, counts, n_units, S_pad


def kernel(reco, target, clabel, batch_index, num_batches, num_clusters):
    from concourse.bass_utils import run_bass_kernel_spmd

    B = int(num_batches)
    C = int(num_clusters)
    reco = np.asarray(reco, dtype=np.float32).reshape(-1)
    target = np.asarray(target, dtype=np.float32).reshape(-1)
    clabel = np.asarray(clabel).reshape(-1)
    batch_index = np.asarray(batch_index).reshape(-1)

    in_maps, unit_seg, counts, n_units, S_pad = _layout(
        reco, target, clabel, batch_index, B, C)

    key = (S_pad,)
    if key not in _prog_cache:
        _prog_cache[key] = _build_program(S_pad)
    nc = _prog_cache[key]

    _last_run["key"] = key
    res = None
    last_err = None
    for _attempt in range(3):  # the device occasionally faults transiently
        try:
            res = run_bass_kernel_spmd(nc, in_maps, list(range(NCORES)))
            break
        except Exception as e:  # noqa: BLE001
            last_err = e
            import time as _time
            _time.sleep(2.0)
    if res is None:
        raise last_err

    colsums = np.concatenate(
        [res.results[m]["out"].reshape(-1) for m in range(NCORES)]
    )[:n_units].astype(np.float64)
    nseg = B * C
    sums = np.bincount(unit_seg, weights=colsums, minlength=nseg)
    cnt = counts.astype(np.float64)

    present = cnt > 0
    means = np.where(present, sums / np.where(present, cnt, 1.0), 0.0)
    means = means.reshape(B, C)
    pmask = present.reshape(B, C).astype(np.float64)
    n_clusters_b = pmask.sum(axis=1)
    b_present = n_clusters_b > 0
    batch_loss = (means * pmask).sum(axis=1) / np.where(b_present, n_clusters_b, 1.0)
    n_b = b_present.sum()
    loss = np.where(b_present, batch_loss, 0.0).sum() / max(n_b, 1)
    return np.float32(loss)


def profile_hw(np_inputs=None, k1=4, k2=1004, pairs=10, verbose=False):
    """Measure steady-state HW ns per kernel iteration.

    Two hardware-loop variants (k1/k2 repeats, Internal-DRAM inputs) run in
    interleaved pairs; median per-pair difference / (k2-k1) cancels dispatch
    overhead and is robust to slow patches on the time-shared device.
    """
    import time
    from concourse.bass_utils import run_bass_kernel_spmd
    if not _last_run and np_inputs is not None:
        kernel(**np_inputs)
    (S_pad,) = _last_run["key"]

    ncs = {}
    for k in (k1, k2):
        ck = ("prof", S_pad, k)
        if ck not in _prog_cache:
            _prog_cache[ck] = _build_program(S_pad, repeat=k,
                                             internal_inputs=True)
        ncs[k] = _prog_cache[ck]

    def one(k):
        t0 = time.time()
        run_bass_kernel_spmd(ncs[k], [{} for _ in range(NCORES)],
                             list(range(NCORES)))
        return time.time() - t0

    one(k1)  # warm both NEFFs
    one(k2)
    diffs = []
    for _ in range(pairs):
        try:
            ta = one(k1)
            tb = one(k2)
        except Exception:  # transient device flake: skip pair
            time.sleep(2)
            continue
        diffs.append((tb - ta) / (k2 - k1) * 1e9)
    diffs.sort()
    if verbose:
        print("pair diffs (ns/iter):", [f"{d:.0f}" for d in diffs])
    return diffs[len(diffs) // 2] if diffs else float("nan")


# revision 47
# speedup vs baseline: 2.3392x; 1.0160x over previous
"""Trainium2 Bass kernel for nn_AutoEncoderLoss (two-level segment-mean MSE).

Strategy
--------
The loss needs per-(batch, cluster) sums of (reco-target)^2 and counts.
Counts depend only on the integer labels, so they are metadata computed on
the host while building the shard layout. For the float work, the host
chooses a *segment-sorted, column-aligned* layout: points are permuted so
each (batch, cluster) segment is contiguous and padded to a multiple of 128
(pad points have reco=target=0). Laid out column-major as [128, S], every
SBUF column then belongs to exactly one segment.

The device kernel is a pure streaming pipeline over [128, 2, S] fp8e4 input
(rec/tar at the middle axis):
  DVE:  v = (rec - tar)^2 in ONE pass via a custom fused DVE op
        (SQDIFF_ANT = sq(Src0 - Src1), registered at build time). This
        keeps ScalarE (measured ~2-3x slower than its cost model) and
        GpSimd (far slower) out of the hot path entirely.
  PE:   ones-stationary matmul per 512-column chunk writes that chunk's
        per-column sums into its own PSUM row: a sliding zeros|ones|zeros
        stationary window routes chunk k to row k. Chunks are split over
        two PSUM tiles so the first block's ScalarE copy-out + DMA overlap
        the second block's matmuls.
The host bincounts the returned column sums into the [B*C] segment buffer
(column -> segment map is host metadata) and does the final masked
two-level mean in float64.

fp8e4 input quantization adds ~0.14% bias to the loss (gate is 2e-2); the
heavy traffic is 2 * 1 byte/point (~17 KB/partition/core), with DVE's
fused pass (~9 us model) and DMA (~7 us model) as the co-pacers.
"""

import numpy as np
import ml_dtypes
from contextlib import ExitStack

NCORES = 8
CHUNK = 512          # PSUM bank columns (fp32) per chunk = one PSUM row
SUB_DVE_FRAC = 0.65  # fraction of sub columns on DVE (rest GpSimd)
SQ_DVE_FRAC = 0.15   # fraction of square columns on DVE (rest ScalarE)
IN_DTYPE = "fp8"     # "fp8" (float8e4) or "bf16" input encoding
DUAL_QUEUE = False   # alternate input DMAs between SP and Activation queues
                     # (measured: SP-only is faster; Act-issued DMAs hurt)
USE_FUSED = True     # single custom-DVE op sq(rec - tar) instead of sub+square
USE_DR = True        # fp8 DoubleRow matmul: each 512-col chunk reduces to 256
                     # pair-sums (cols n, n+256 same segment by host layout);
                     # 4x fewer PE cycles. Requires stationary M multiple of 16.


def _register_sqdiff():
    """Register (once) the custom DVE op out = (in0 - in1)^2."""
    import concourse.dve_ops as dve_ops
    from concourse.dve_spec import Spec, Src0, Src1, sq, lower
    from concourse.dve_uop import DveOpSpec

    for op in dve_ops.OPS:
        if op.name == "SQDIFF_ANT":
            return op
    spec = Spec(body=sq(Src0 - Src1),
                reference=lambda in0, in1: (in0 - in1) ** 2)
    row = dve_ops._CUSTOM_DVE_ROW_BASE + len(dve_ops.OPS)
    assert row < 0x20
    shas = {}
    for ver in ("v3", "v4"):
        s = DveOpSpec(name="SQDIFF_ANT", opcode=row,
                      uops=lower(spec, ver=ver), rd1_en=True)
        shas[ver] = s.sha(ver)
    op = dve_ops.DveOp("SQDIFF_ANT", spec, subdim=False, uops_sha=shas)
    dve_ops.OPS.append(op)
    dve_ops._SUB_OPCODE_FOR_NAME[op.name] = row
    dve_ops.CUSTOM_DVE_SPECS[op.name] = op.spec
    return op

_prog_cache = {}
_last_run = {}


T_TILE = 2048        # main streaming tile width (columns)


def _tiles_for(S_pad):
    """Tile widths: small head (fill pipeline fast), big middle, small tail
    (short trailing compute chain). All multiples of CHUNK."""
    tiles = []
    rem = S_pad
    head = min(512, rem)
    tiles.append(head)
    rem -= head
    while rem > T_TILE + 1024:
        tiles.append(T_TILE)
        rem -= T_TILE
    while rem > 512:
        w = min(1024, rem - 512)
        w = max(w // CHUNK, 1) * CHUNK
        tiles.append(w)
        rem -= w
    if rem:
        tiles.append(rem)
    out = []
    t0 = 0
    for w in tiles:
        out.append((t0, w))
        t0 += w
    assert t0 == S_pad
    return out


def _build_program(S_pad, repeat=None, internal_inputs=False, stage="full"):
    """SPMD program: [128, 2, S_pad] fp8e4 -> [n_chunks, 512] f32 col sums.

    stage: "dma" (loads only), "dve" (+fused square-diff), "pe" (+matmuls),
    "full" (+copy-out). Cut-down stages are for bottleneck isolation.
    """
    import concourse.tile as tile
    from concourse import bacc, mybir

    f32 = mybir.dt.float32
    bf16 = mybir.dt.bfloat16
    in_dt = mybir.dt.float8e4 if IN_DTYPE == "fp8" else bf16
    AT = mybir.ActivationFunctionType

    assert S_pad % CHUNK == 0
    nch = S_pad // CHUNK
    assert nch <= 128

    sqdiff = _register_sqdiff() if USE_FUSED else None
    nc = bacc.Bacc("TRN2", target_bir_lowering=False, debug=False,
                   num_devices=NCORES)
    in_kind = "Internal" if internal_inputs else "ExternalInput"
    dat = nc.dram_tensor("dat", [128, 2, S_pad], in_dt, kind=in_kind).ap()
    outw = CHUNK // 2 if USE_DR else CHUNK
    out = nc.dram_tensor("out", [nch, outw], f32, kind="ExternalOutput").ap()
    if USE_DR:
        assert IN_DTYPE == "fp8" and USE_FUSED

    tiles = _tiles_for(S_pad)

    with tile.TileContext(nc) as tc, ExitStack() as ctx:
        io_pool = ctx.enter_context(tc.tile_pool(name="io", bufs=4))
        tmp_pool = ctx.enter_context(tc.tile_pool(name="tmp", bufs=3))
        psum_pool = ctx.enter_context(tc.tile_pool(name="ps", bufs=2, space="PSUM"))
        const_pool = ctx.enter_context(tc.tile_pool(name="cst", bufs=1))

        # zeros|ones|zeros constant: sliding window k has its ones-column at
        # position k, routing chunk k's column sums to PSUM row k. (A plain
        # [128,1] ones stationary into ps[k:k+1] is not possible: PE tile
        # col offsets are restricted to {0,32,64,96}.) Chunks are split over
        # two PSUM tiles so the first block's copy-out overlaps the second
        # block's matmuls.
        use_pe = stage in ("pe", "full")
        if USE_DR:
            # DoubleRow: stationary must be [128, 2, M] fp8 with M a multiple
            # of 16 (ISA restriction); fixed M=16 blocks, unused rows just
            # accumulate zeros. Output per chunk = 256 pair-sums.
            M = 16
            assert nch <= 2 * M
            nchA = min(nch, M)
            nchB = nch - nchA
            OUTW = CHUNK // 2
            # one aligned [128, 2, M] stationary per row k (sliding-window
            # slices at 1-byte offsets fail s3_lw_dual_fp8_restrictions)
            ones_buf = const_pool.tile([128, M, 2, M], in_dt, tag="ones")
            nc.vector.memset(ones_buf[:], 0.0)
            for kk in range(M):
                nc.vector.memset(ones_buf[:, kk, :, kk:kk + 1], 1.0)
            psA = (psum_pool.tile([M, OUTW], f32, tag="psA", name="psA")
                   if use_pe else None)
            psB = (psum_pool.tile([M, OUTW], f32, tag="psB", name="psB")
                   if use_pe and nchB else None)
        else:
            nchA = (nch + 1) // 2
            nchB = nch - nchA
            OUTW = CHUNK
            W = 2 * max(nchA, nchB) - 1
            ones_buf = const_pool.tile([128, W], bf16, tag="ones")
            nc.vector.memset(ones_buf[:], 0.0)
            nc.vector.memset(
                ones_buf[:, max(nchA, nchB) - 1:max(nchA, nchB)], 1.0)
            psA = (psum_pool.tile([nchA, OUTW], f32, tag="psA", name="psA")
                   if use_pe else None)
            psB = (psum_pool.tile([nchB, OUTW], f32, tag="psB", name="psB")
                   if use_pe and nchB else None)

        if repeat is not None:
            ctx.enter_context(tc.For_i(0, repeat, 1))

        k = 0
        for i, (t0, tw) in enumerate(tiles):
            q = nc.sync if (i % 2 == 0 or not DUAL_QUEUE) else nc.scalar
            dt_ = io_pool.tile([128, 2, tw], in_dt, tag="dat")
            q.dma_start(out=dt_[:], in_=dat[:, :, t0:t0 + tw])
            if stage == "dma":
                continue

            v = tmp_pool.tile([128, tw], in_dt if USE_DR else bf16, tag="v")
            if USE_FUSED:
                nc.vector._custom_dve(sqdiff, out=v[:], in0=dt_[:, 0, :],
                                      in1=dt_[:, 1, :])
            else:
                w1 = min(int(tw * SUB_DVE_FRAC / 16) * 16, tw)
                d = tmp_pool.tile([128, tw], bf16, tag="d")
                if w1:
                    nc.vector.tensor_sub(d[:, :w1], dt_[:, 0, :w1],
                                         dt_[:, 1, :w1])
                if w1 < tw:
                    nc.gpsimd.tensor_sub(d[:, w1:], dt_[:, 0, w1:],
                                         dt_[:, 1, w1:])

                w2 = min(int(tw * SQ_DVE_FRAC / 16) * 16, tw)
                if w2:
                    nc.vector.tensor_mul(v[:, :w2], d[:, :w2], d[:, :w2])
                if w2 < tw:
                    nc.scalar.activation(v[:, w2:], d[:, w2:], AT.Square)

            if stage == "dve":
                continue
            for j in range(tw // CHUNK):
                if k < nchA:
                    ps, kk, first, last = psA, k, k == 0, k == nchA - 1
                else:
                    ps, kk, first, last = (
                        psB, k - nchA, k == nchA, k == nch - 1)
                if USE_DR:
                    lhsT = ones_buf[:, kk, :, :]  # aligned [128, 2, 16]
                    rhs = v[:, j * CHUNK:(j + 1) * CHUNK].rearrange(
                        "p (j2 n) -> p j2 n", j2=2)
                    nc.tensor.matmul(
                        ps[:], lhsT, rhs, start=first, stop=last,
                        perf_mode=mybir.MatmulPerfMode.DoubleRow)
                else:
                    n_blk = nchA if k < nchA else nchB
                    base = max(nchA, nchB) - 1
                    lhsT = ones_buf[:, base - kk:base - kk + n_blk]
                    nc.tensor.matmul(ps[:], lhsT,
                                     v[:, j * CHUNK:(j + 1) * CHUNK],
                                     start=first, stop=last)
                k += 1

        if stage == "full":
            cp = nc.scalar if USE_FUSED else nc.vector  # ScalarE idle if fused
            copy = cp.copy if USE_FUSED else cp.tensor_copy
            obA = const_pool.tile([nchA, outw], f32, tag="obA")
            copy(obA[:], psA[:nchA, :])  # fires once chunk nchA-1 is done
            nc.sync.dma_start(out=out[:nchA, :], in_=obA[:])
            if nchB:
                obB = const_pool.tile([nchB, outw], f32, tag="obB")
                copy(obB[:], psB[:nchB, :])
                nc.sync.dma_start(out=out[nchA:, :], in_=obB[:])

    nc.compile()
    return nc


def _layout(reco, target, clabel, batch_index, B, C):
    """Segment-sorted column-aligned shard layout (all host metadata work).

    Returns per-core fp8 [128, 2, S_pad] buffers, the column->segment map,
    exact per-segment counts, and S_total/S_pad.
    """
    N = reco.shape[0]
    seg = (batch_index.astype(np.int32) * np.int32(C)
           + clabel.astype(np.int32))
    nseg = B * C
    counts = np.bincount(seg, minlength=nseg)
    if USE_DR:
        # DoubleRow merges column pairs (n, n+256) of each 512-col chunk, so
        # segments must cover an even number of columns (256-point multiples).
        pad_cols = 2 * ((counts + 255) // 256)
    else:
        pad_cols = (counts + 127) // 128        # columns per segment
    col_start = np.zeros(nseg, dtype=np.int64)
    np.cumsum(pad_cols[:-1], out=col_start[1:])
    S_total = int(pad_cols.sum())

    S_core = -(-S_total // NCORES)
    S_pad = -(-S_core // CHUNK) * CHUNK
    S_cap = NCORES * S_pad

    # stable counting sort by segment; rank of each point within its segment
    perm = np.argsort(seg, kind="stable")
    pt_start = np.zeros(nseg, dtype=np.int64)
    np.cumsum(counts[:-1], out=pt_start[1:])
    rank = np.empty(N, dtype=np.int64)
    rank[perm] = np.arange(N, dtype=np.int64) - np.repeat(pt_start, counts)
    dest = 128 * col_start[seg] + rank          # linear slot, column-major

    buf = np.zeros((2, S_cap * 128), dtype=np.float32)
    buf[0, dest] = reco
    buf[1, dest] = target
    # [2, S_cap, 128] -> [128, 2, S_cap], contiguous per core after slicing
    np_dt = (ml_dtypes.float8_e4m3fn if IN_DTYPE == "fp8"
             else ml_dtypes.bfloat16)
    arr = np.ascontiguousarray(
        buf.reshape(2, S_cap, 128).transpose(2, 0, 1)
    ).astype(np_dt)

    if USE_DR:
        # logical col l = c*512 + 2m + j  ->  physical c*512 + j*256 + m, so
        # the device's pair (n, n+256) is the logical pair (2m, 2m+1).
        arr = np.ascontiguousarray(
            arr.reshape(128, 2, S_cap // CHUNK, CHUNK // 2, 2)
            .swapaxes(-1, -2)
            .reshape(128, 2, S_cap))
        unit_seg = np.repeat(np.arange(nseg, dtype=np.int64), pad_cols // 2)
        n_units = S_total // 2
    else:
        unit_seg = np.repeat(np.arange(nseg, dtype=np.int64), pad_cols)
        n_units = S_total

    in_maps = []
    for m in range(NCORES):
        dat = np.ascontiguousarray(arr[:, :, m * S_pad:(m + 1) * S_pad])
        in_maps.append({"dat": dat})
    return in_maps, unit_seg, counts, n_units, S_pad


def kernel(reco, target, clabel, batch_index, num_batches, num_clusters):
    from concourse.bass_utils import run_bass_kernel_spmd

    B = int(num_batches)
    C = int(num_clusters)
    reco = np.asarray(reco, dtype=np.float32).reshape(-1)
    target = np.asarray(target, dtype=np.float32).reshape(-1)
    clabel = np.asarray(clabel).reshape(-1)
    batch_index = np.asarray(batch_index).reshape(-1)

    in_maps, unit_seg, counts, n_units, S_pad = _layout(
        reco, target, clabel, batch_index, B, C)

    key = (S_pad,)
    if key not in _prog_cache:
        _prog_cache[key] = _build_program(S_pad)
    nc = _prog_cache[key]

    _last_run["key"] = key
    res = None
    last_err = None
    for _attempt in range(3):  # the device occasionally faults transiently
        try:
            res = run_bass_kernel_spmd(nc, in_maps, list(range(NCORES)))
            break
        except Exception as e:  # noqa: BLE001
            last_err = e
            import time as _time
            _time.sleep(2.0)
    if res is None:
        raise last_err

    colsums = np.concatenate(
        [res.results[m]["out"].reshape(-1) for m in range(NCORES)]
    )[:n_units].astype(np.float64)
    nseg = B * C
    sums = np.bincount(unit_seg, weights=colsums, minlength=nseg)
    cnt = counts.astype(np.float64)

    present = cnt > 0
    means = np.where(present, sums / np.where(present, cnt, 1.0), 0.0)
    means = means.reshape(B, C)
    pmask = present.reshape(B, C).astype(np.float64)
    n_clusters_b = pmask.sum(axis=1)
    b_present = n_clusters_b > 0
    batch_loss = (means * pmask).sum(axis=1) / np.where(b_present, n_clusters_b, 1.0)
    n_b = b_present.sum()
    loss = np.where(b_present, batch_loss, 0.0).sum() / max(n_b, 1)
    return np.float32(loss)


def profile_hw(np_inputs=None, k1=4, k2=1004, pairs=10, verbose=False):
    """Measure steady-state HW ns per kernel iteration.

    Two hardware-loop variants (k1/k2 repeats, Internal-DRAM inputs) run in
    interleaved pairs; median per-pair difference / (k2-k1) cancels dispatch
    overhead and is robust to slow patches on the time-shared device.
    """
    import time
    from concourse.bass_utils import run_bass_kernel_spmd
    if not _last_run and np_inputs is not None:
        kernel(**np_inputs)
    (S_pad,) = _last_run["key"]

    ncs = {}
    for k in (k1, k2):
        ck = ("prof", S_pad, k)
        if ck not in _prog_cache:
            _prog_cache[ck] = _build_program(S_pad, repeat=k,
                                             internal_inputs=True)
        ncs[k] = _prog_cache[ck]

    def one(k):
        t0 = time.time()
        run_bass_kernel_spmd(ncs[k], [{} for _ in range(NCORES)],
                             list(range(NCORES)))
        return time.time() - t0

    one(k1)  # warm both NEFFs
    one(k2)
    diffs = []
    for _ in range(pairs):
        try:
            ta = one(k1)
            tb = one(k2)
        except Exception:  # transient device flake: skip pair
            time.sleep(2)
            continue
        diffs.append((tb - ta) / (k2 - k1) * 1e9)
    diffs.sort()
    if verbose:
        print("pair diffs (ns/iter):", [f"{d:.0f}" for d in diffs])
    return diffs[len(diffs) // 2] if diffs else float("nan")
